# revision 20
# baseline (speedup 1.0000x reference)
"""CoefficientMaxPool Trainium2 kernel (8-core data-parallel), v8.

Problem: x [32, 512, 16, 128] f32.  Irreps group into degree blocks
l=0:[0,1), l=1:[1,4), l=2:[4,9), l=3:[9,16).  Per (batch, l, channel):
find the neighbor n* maximizing the degree-block squared norm, output
that neighbor's block components -> out [32, 16, 128].

Per core (4 batches), per batch, layout X [p=128(n%128), a=4, i=16, c=128]:
  ACT : X2 = X*X fp32 (squares first -- they gate the DVE norms; ACT
        runs one batch ahead: sq(b+1) before cp(b))
  ACT : Xbf = bf16 copy of X (feeds the 2x-mode select, needed late)
  DVE : block norms via 5-op strided-AP add tree, in place into X2
        slots {3,6,9} -> norm slots {0,3,6,9}, a single stride-3 set
        (fp32 exact -- required for winner uniqueness)
  DVE : a-max via 2 ops (pairwise over a, join) -> M1[p,l,c] l-ordered
  GPS : GM = partition_all_reduce(max) over M1 (bcast to all partitions)
  DVE : mask[p,a,l,c] = is_equal(norm, GM) bf16, ONE op (stride-3 src)
  DVE : Xbf *= mask[l(i)] in place, bf16 all-SBUF step-1 -> 2x_1P;
        order l3,l2,l1,l0 so the PE accumulate chain drains early
  PE  : ps[row 32k] += ones^T @ Xbf-chunk-k -- PSUM rows 0/32/64/96,
        each k its own accumulation group (enables split flush on b3)
  ACT : ps rows -> SBUF ob[4,512] (short copy), DMA out.

History: v1 127.8us (DVE 118us busy, 164 TT ops) -> v4 107.7 (strided-AP
op trees, 2x bf16 select, gpsimd global-max, PSUM row-spread) -> v5
104.5 (DMAs up front, sq before cp, fill 29->19us) -> v7 103.4 (ACT one
batch ahead).  v7 analysis: DVE ops now match (N+151)/0.96 EXACTLY (no
strided-row penalty); 2.2us/batch DVE stall before mask(b) because the
Tile scheduler models partition_all_reduce at ~0.7us (real 2.05us) and
parks mask right after amax; fill ~19us; tail ~7us.
v8: calibrate the scheduler's gpsimd efficiency so it fills the GM
latency with tree(b+1), norm slots {0,3,6,9} (amax 3->2 ops, mask 2->1),
b0 quarter-trees, split flush for the last batch.

Hard constraints learned (do not re-derive):
- fp32 TT on DVE measures (N+151)/0.96 ns when the pipeline is clean;
  bf16 SBUF step-1 TT is 2x ((N/2+151)/0.96); any PSUM operand -> 1x.
  ACT ACTIVATE ~0.9ns/elem + ~330ns/op.  The Tile scheduler REORDERS
  freely (emission order is only a priority hint); fix its cost model
  rather than fighting emission order.
- norms must be exact fp32 end-to-end: bf16 norms create argmax ties
  across neighbors -> two mask hits -> summed output -> FAIL.
- gpsimd/Pool CANNOT run TensorTensor/TensorScalar: nc.compile() passes
  but neuronxcc walrus codegen fails at NEFF-build time ("Instruction
  engine check failed (Pool)", re-verified v6).  Only memset +
  custom-ucode lib ops run there.  partition_all_reduce [128p,512f]
  takes ~2.05us on HW (efficiency ~0.21 vs Pool roofline).
- bf16 memset at 2B-aligned (non-4B) SBUF offsets crashes the device
  (NRT_EXEC_UNIT_UNRECOVERABLE): build constants in fp32 + ACT cast.
- DMA cannot touch PSUM; PSUM->SBUF copies go on ACT or DVE.
- winner-select can't be a PE matmul (per-channel diagonal extraction).
- PE matmul out free <= 512 fp32 (one PSUM bank); matmul() self-loads
  weights; out base partition must be 0/32/64/96.
- runtime preamble delays the first input-DMA packet to ~9us.
"""

import os
import sys

import numpy as np

for _p in ("/opt/trn_rl_repo", "/opt/pypackages"):
    if _p not in sys.path:
        sys.path.append(_p)

from contextlib import ExitStack

import concourse.bacc as bacc
import concourse.bass as bass
import concourse.bass_isa as bass_isa
import concourse.hw_specs as hw_specs
import concourse.tile as tile
from concourse import library_config, mybir

# The legacy CoreSim Tile scheduler prices a gpsimd custom op with cost
# model v1: free_size x CYCLE_T[Pool] (~427ns for [128,512]) -- the
# GPSIMD_IMPL_EFFICIENCY table is loaded but unused there.  Real
# partition_all_reduce is ~2.05us, so the scheduler parks mask(b) right
# behind amax(b) and the in-order DVE queue idles ~2.2us per batch.
# Calibrate by slowing the modeled Pool clock ~4.8x; the only Pool work
# in this kernel is partition_all_reduce.
hw_specs.TRN2Spec.CYCLE_T = {
    **hw_specs.TRN2Spec.CYCLE_T,
    mybir.EngineType.Pool: 1e9 / 0.25e9,
}
hw_specs.TRN2Spec.GPSIMD_IMPL_EFFICIENCY = {
    **hw_specs.TRN2Spec.GPSIMD_IMPL_EFFICIENCY,
    "PartitionAllReduce": 0.21,
}

N_CORES = 8
B_FULL, N, IRR, C = 32, 512, 16, 128
B = B_FULL // N_CORES  # 4 batches per core
P = 128                # partitions (n within chunk)
A = N // P             # 4 neighbor chunks
F32 = mybir.dt.float32
BF16 = mybir.dt.bfloat16
ADD = mybir.AluOpType.add
MAX = mybir.AluOpType.max
MULT = mybir.AluOpType.mult
EQ = mybir.AluOpType.is_equal

_cache = {}


def _build_bass():
    nc = bacc.Bacc("TRN2", target_bir_lowering=False, debug=False,
                   num_devices=N_CORES)
    x_in = nc.dram_tensor("x", [B, N, IRR, C], F32, kind="ExternalInput")
    out_t = nc.dram_tensor("out", [B, IRR, C], F32, kind="ExternalOutput")

    with tile.TileContext(nc) as tc, ExitStack() as ctx:
        # DRAM view: n = a*P + p  ->  [b, p, a, i, c]
        x_v = x_in.ap().rearrange("b (a p) i c -> b p a i c", p=P)
        out_kv = out_t.ap().rearrange("b i c -> b (i c)").rearrange(
            "b (k f) -> b k f", k=4)

        xp = ctx.enter_context(tc.tile_pool(name="xp", bufs=2))
        x2p = ctx.enter_context(tc.tile_pool(name="x2p", bufs=2))
        xbp = ctx.enter_context(tc.tile_pool(name="xbp", bufs=2))
        med = ctx.enter_context(tc.tile_pool(name="med", bufs=2))
        # M1 is read by gpsimd, GM written by it; triple-buffer so later
        # batches' DVE writes don't WAR-stall on gpsimd reads.
        gmp = ctx.enter_context(tc.tile_pool(name="gmp", bufs=3))
        obp = ctx.enter_context(tc.tile_pool(name="obp", bufs=4))
        singles = ctx.enter_context(tc.tile_pool(name="singles", bufs=1))
        pout = ctx.enter_context(tc.tile_pool(name="pout", bufs=2,
                                              space="PSUM"))

        # gpsimd library providing InstPartitionAllReduce
        nc.gpsimd.load_library(library_config.attn)

        # W4[:, k, :]: bf16 stationary whose only nonzero column is k ->
        # matmul writes chunk-k's neighbor-sum into PSUM partition row k
        # (rows 0-3 contiguous: engine APs require partition step 1, and
        # matmul out base partition must be 0/32/64).  Built in fp32
        # (bf16 memset at 2B-aligned offsets crashes the HW), ACT-cast.
        W4f = singles.tile([P, 4, 4], F32)
        nc.vector.memset(W4f, 0.0)
        for k in range(4):
            nc.vector.memset(W4f[:, k, k:k + 1], 1.0)
        W4 = singles.tile([P, 4, 4], BF16)
        nc.scalar.copy(out=W4, in_=W4f)
        # Prewarm the ACT Square table (~1.3us) before real data arrives
        warm = singles.tile([P, 1], F32)
        nc.vector.memset(warm, 0.0)
        nc.scalar.activation(warm, warm, mybir.ActivationFunctionType.Square)

        def chunks_of(b):
            nq = 4 if b == 0 else 2
            step = A // nq
            return [slice(step * q, step * (q + 1)) for q in range(nq)]

        def load_dma(b):
            X = xp.tile([P, A, IRR, C], F32, tag="X")
            X2 = x2p.tile([P, A, IRR, C], F32, tag="X2")
            Xbf = xbp.tile([P, A, IRR, C], BF16, tag="Xbf")
            for ha in chunks_of(b):
                nc.sync.dma_start(out=X[:, ha], in_=x_v[b][:, ha])
            return X, X2, Xbf

        def do_sq(b, t):
            X, X2, _ = t
            for ha in chunks_of(b):
                nc.scalar.activation(X2[:, ha], X[:, ha],
                                     mybir.ActivationFunctionType.Square)

        def do_cp(b, t):
            X, _, Xbf = t
            if b == 0:
                # fill phase: SDMA is saturated streaming inputs; keep the
                # cast off the DMA rings
                for ha in chunks_of(b):
                    nc.scalar.copy(out=Xbf[:, ha], in_=X[:, ha])
            else:
                # half on ACT, half as a SWDGE cast-DMA (fp32->bf16 during
                # SBUF->SBUF DMA; only gpsimd DGEs can cast).  Frees ~3.7us
                # of ACT per batch so sq(b+1) lands before the DVE needs it.
                nc.scalar.copy(out=Xbf[:, 0:2], in_=X[:, 0:2])
                nc.gpsimd.dma_start(out=Xbf[:, 2:4], in_=X[:, 2:4])

        def norm_tree(X2, aa):
            """Block norms in place into X2 slots {3,6,9} (l1,l2,l3; l0
            stays slot 0) via a 5-op strided add tree (12 pairwise adds)
            over a-chunk slice `aa`:
              op1: {2,4,6}+={3,5,7}    op2: {10,12,14}+={11,13,15}
              op3: {3,6} = {1,4}+{2,6}  op4: {9,12}+={10,14}
              op5: {6,9}+={8,12}
            """
            def tadd(dst, src):
                nc.vector.tensor_tensor(dst, dst, src, ADD)

            tadd(X2[:, aa, 2:8:2, :], X2[:, aa, 3:9:2, :])
            tadd(X2[:, aa, 10:16:2, :], X2[:, aa, 11:16:2, :])
            nc.vector.tensor_tensor(X2[:, aa, 3:7:3, :],
                                    X2[:, aa, 1:5:3, :],
                                    X2[:, aa, 2:7:4, :], ADD)
            tadd(X2[:, aa, 9:13:3, :], X2[:, aa, 10:15:4, :])
            tadd(X2[:, aa, 6:10:3, :], X2[:, aa, 8:13:4, :])

        def stage1(b, t):
            """Norm tree, a-max, GM."""
            _, X2, _ = t
            if b == 0:
                # per-quarter trees so batch 0's norms start right after
                # quarter 0's square
                for q in range(A):
                    norm_tree(X2, slice(q, q + 1))
            else:
                norm_tree(X2, slice(0, A))

            # a-max: pairwise over a (0,2),(1,3) then join; norm slots
            # {0,3,6,9} are a single stride-3 AP.
            R = med.tile([P, 2, 4, C], F32, tag="R")
            nc.vector.tensor_tensor(R, X2[:, 0:2, 0:10:3, :],
                                    X2[:, 2:4, 0:10:3, :], MAX)
            M1 = gmp.tile([P, 4, C], F32, tag="M1")
            nc.vector.tensor_tensor(M1, R[:, 0], R[:, 1], MAX)

            # Global max over the 128 partitions, broadcast to all.  [GPSIMD]
            GM = gmp.tile([P, 4, C], F32, tag="GM")
            nc.gpsimd.partition_all_reduce(
                GM.rearrange("p l c -> p (l c)"),
                M1.rearrange("p l c -> p (l c)"),
                channels=P, reduce_op=bass_isa.ReduceOp.max)
            return GM

        def stage2(b, t, GM):
            """Mask, in-place winner-select, PE reduce."""
            _, X2, Xbf = t
            mask = med.tile([P, A, 4, C], BF16, tag="mask")
            nc.vector.tensor_tensor(
                mask, X2[:, :, 0:10:3, :],
                GM.unsqueeze(1).broadcast_to([P, A, 4, C]), EQ)

            Xf = Xbf.rearrange("p a i c -> p a (i c)")
            ps = pout.tile([4, 512], F32, tag="ps")

            def sel(s, e, l):
                nc.vector.tensor_tensor(
                    Xbf[:, :, s:e, :], Xbf[:, :, s:e, :],
                    mask[:, :, l, :].unsqueeze(2).broadcast_to(
                        [P, A, e - s, C]), MULT)

            def mm(k, start=False, stop=False):
                # all 16 matmuls form one accumulation group on ps[4,512]
                # (W4_k zeroes the other rows; every matmul writes all 4)
                for a in range(A):
                    nc.tensor.matmul(ps, W4[:, k, :],
                                     Xf[:, a, k * 512:(k + 1) * 512],
                                     start=(start and a == 0),
                                     stop=(stop and a == A - 1))

            # l3 first so the PE accumulate chains drain early
            sel(9, 16, 3)  # l3: i 9-15
            mm(3, start=True)       # k3 needs i 12-15
            sel(4, 9, 2)   # l2: i 4-8
            mm(2)          # k2 needs i 8-11
            mm(1)          # k1 needs i 4-7
            sel(1, 4, 1)   # l1: i 1-3
            sel(0, 1, 0)   # l0: i 0
            mm(0, stop=True)        # k0 needs i 0-3
            return ps

        def flush(b, ps):
            """PSUM rows 0-3 -> SBUF [4,512] -> DRAM."""
            ob = obp.tile([4, 512], F32, tag="ob")
            nc.scalar.copy(out=ob, in_=ps)
            nc.sync.dma_start(out=out_kv[b], in_=ob)

        # Software pipeline.  ACT runs one batch ahead on squares
        # (sq(b+1) before cp(b)); DVE runs tree(b+1) between amax(b) and
        # mask(b) so the gpsimd global-max latency is hidden.
        tl = {0: load_dma(0), 1: load_dma(1)}
        do_sq(0, tl[0])
        gm = {0: stage1(0, tl[0])}
        do_sq(1, tl[1])
        do_cp(0, tl[0])
        for b in range(B):
            if b + 2 < B:
                tl[b + 2] = load_dma(b + 2)
            if b + 1 < B:
                gm[b + 1] = stage1(b + 1, tl[b + 1])
            if b + 2 < B:
                do_sq(b + 2, tl[b + 2])
            if b + 1 < B:
                do_cp(b + 1, tl[b + 1])
            flush(b, stage2(b, tl.pop(b), gm.pop(b)))

    nc.compile()
    return nc


def kernel(x: np.ndarray, i2l: np.ndarray | None = None) -> np.ndarray:
    x = np.ascontiguousarray(np.asarray(x), dtype=np.float32)
    assert x.shape == (B_FULL, N, IRR, C), x.shape

    if "nc" not in _cache:
        _cache["nc"] = _build_bass()
    nc = _cache["nc"]

    from concourse.bass_utils import run_bass_kernel_spmd

    in_maps = [{"x": x[i * B:(i + 1) * B]} for i in range(N_CORES)]
    res = run_bass_kernel_spmd(nc, in_maps, list(range(N_CORES)))
    out = np.concatenate([res.results[i]["out"] for i in range(N_CORES)], axis=0)
    return out


if __name__ == "__main__":
    xs = np.random.randn(B_FULL, N, IRR, C).astype(np.float32)
    o = kernel(xs)
    print("out", o.shape, o.dtype)


# revision 22
# speedup vs baseline: 1.0796x; 1.0796x over previous
"""CoefficientMaxPool Trainium2 kernel (8-core data-parallel), v8.

Problem: x [32, 512, 16, 128] f32.  Irreps group into degree blocks
l=0:[0,1), l=1:[1,4), l=2:[4,9), l=3:[9,16).  Per (batch, l, channel):
find the neighbor n* maximizing the degree-block squared norm, output
that neighbor's block components -> out [32, 16, 128].

Per core (4 batches), per batch, layout X [p=128(n%128), a=4, i=16, c=128]:
  ACT : X2 = X*X fp32 (squares first -- they gate the DVE norms; ACT
        runs one batch ahead: sq(b+1) before cp(b))
  ACT : Xbf = bf16 copy of X (feeds the 2x-mode select, needed late)
  DVE : block norms via 5-op strided-AP add tree, in place into X2
        slots {3,6,9} -> norm slots {0,3,6,9}, a single stride-3 set
        (fp32 exact -- required for winner uniqueness)
  DVE : a-max via 2 ops (pairwise over a, join) -> M1[p,l,c] l-ordered
  GPS : GM = partition_all_reduce(max) over M1 (bcast to all partitions)
  DVE : mask[p,a,l,c] = is_equal(norm, GM) bf16, ONE op (stride-3 src)
  DVE : Xbf *= mask[l(i)] in place, bf16 all-SBUF step-1 -> 2x_1P;
        order l3,l2,l1,l0 so the PE accumulate chain drains early
  PE  : ps[row 32k] += ones^T @ Xbf-chunk-k -- PSUM rows 0/32/64/96,
        each k its own accumulation group (enables split flush on b3)
  ACT : ps rows -> SBUF ob[4,512] (short copy), DMA out.

History: v1 127.8us (DVE 118us busy, 164 TT ops) -> v4 107.7 (strided-AP
op trees, 2x bf16 select, gpsimd global-max, PSUM row-spread) -> v5
104.5 (DMAs up front, sq before cp, fill 29->19us) -> v7 103.4 (ACT one
batch ahead).  v7 analysis: DVE ops now match (N+151)/0.96 EXACTLY (no
strided-row penalty); 2.2us/batch DVE stall before mask(b) because the
Tile scheduler models partition_all_reduce at ~0.7us (real 2.05us) and
parks mask right after amax; fill ~19us; tail ~7us.
v8: calibrate the scheduler's gpsimd efficiency so it fills the GM
latency with tree(b+1), norm slots {0,3,6,9} (amax 3->2 ops, mask 2->1),
b0 quarter-trees, split flush for the last batch.

Hard constraints learned (do not re-derive):
- fp32 TT on DVE measures (N+151)/0.96 ns when the pipeline is clean;
  bf16 SBUF step-1 TT is 2x ((N/2+151)/0.96); any PSUM operand -> 1x.
  ACT ACTIVATE ~0.9ns/elem + ~330ns/op.  The Tile scheduler REORDERS
  freely (emission order is only a priority hint); fix its cost model
  rather than fighting emission order.
- norms must be exact fp32 end-to-end: bf16 norms create argmax ties
  across neighbors -> two mask hits -> summed output -> FAIL.
- gpsimd/Pool CANNOT run TensorTensor/TensorScalar: nc.compile() passes
  but neuronxcc walrus codegen fails at NEFF-build time ("Instruction
  engine check failed (Pool)", re-verified v6).  Only memset +
  custom-ucode lib ops run there.  partition_all_reduce [128p,512f]
  takes ~2.05us on HW (efficiency ~0.21 vs Pool roofline).
- bf16 memset at 2B-aligned (non-4B) SBUF offsets crashes the device
  (NRT_EXEC_UNIT_UNRECOVERABLE): build constants in fp32 + ACT cast.
- DMA cannot touch PSUM; PSUM->SBUF copies go on ACT or DVE.
- winner-select can't be a PE matmul (per-channel diagonal extraction).
- PE matmul out free <= 512 fp32 (one PSUM bank); matmul() self-loads
  weights; out base partition must be 0/32/64/96.
- runtime preamble delays the first input-DMA packet to ~9us.
"""

import os
import sys

import numpy as np

for _p in ("/opt/trn_rl_repo", "/opt/pypackages"):
    if _p not in sys.path:
        sys.path.append(_p)

from contextlib import ExitStack

import concourse.bacc as bacc
import concourse.bass as bass
import concourse.bass_isa as bass_isa
import concourse.hw_specs as hw_specs
import concourse.tile as tile
from concourse import library_config, mybir

# The legacy CoreSim Tile scheduler prices a gpsimd custom op with cost
# model v1: free_size x CYCLE_T[Pool] (~427ns for [128,512]) -- the
# GPSIMD_IMPL_EFFICIENCY table is loaded but unused there.  Real
# partition_all_reduce is ~2.05us, so the scheduler parks mask(b) right
# behind amax(b) and the in-order DVE queue idles ~2.2us per batch.
# Calibrate by slowing the modeled Pool clock ~4.8x; the only Pool work
# in this kernel is partition_all_reduce.
hw_specs.TRN2Spec.CYCLE_T = {
    **hw_specs.TRN2Spec.CYCLE_T,
    mybir.EngineType.Pool: 1e9 / 0.25e9,
}
hw_specs.TRN2Spec.GPSIMD_IMPL_EFFICIENCY = {
    **hw_specs.TRN2Spec.GPSIMD_IMPL_EFFICIENCY,
    "PartitionAllReduce": 0.21,
}

N_CORES = 8
B_FULL, N, IRR, C = 32, 512, 16, 128
B = B_FULL // N_CORES  # 4 batches per core
P = 128                # partitions (n within chunk)
A = N // P             # 4 neighbor chunks
F32 = mybir.dt.float32
BF16 = mybir.dt.bfloat16
ADD = mybir.AluOpType.add
MAX = mybir.AluOpType.max
MULT = mybir.AluOpType.mult
EQ = mybir.AluOpType.is_equal

_cache = {}


def _build_bass():
    nc = bacc.Bacc("TRN2", target_bir_lowering=False, debug=False,
                   num_devices=N_CORES)
    x_in = nc.dram_tensor("x", [B, N, IRR, C], F32, kind="ExternalInput")
    out_t = nc.dram_tensor("out", [B, IRR, C], F32, kind="ExternalOutput")

    with tile.TileContext(nc) as tc, ExitStack() as ctx:
        # DRAM view: n = a*P + p  ->  [b, p, a, i, c]
        x_v = x_in.ap().rearrange("b (a p) i c -> b p a i c", p=P)
        out_kv = out_t.ap().rearrange("b i c -> b (i c)").rearrange(
            "b (k f) -> b k f", k=4)

        xp = ctx.enter_context(tc.tile_pool(name="xp", bufs=2))
        x2p = ctx.enter_context(tc.tile_pool(name="x2p", bufs=2))
        xbp = ctx.enter_context(tc.tile_pool(name="xbp", bufs=2))
        med = ctx.enter_context(tc.tile_pool(name="med", bufs=2))
        # M1 is read by gpsimd, GM written by it; triple-buffer so later
        # batches' DVE writes don't WAR-stall on gpsimd reads.
        gmp = ctx.enter_context(tc.tile_pool(name="gmp", bufs=3))
        obp = ctx.enter_context(tc.tile_pool(name="obp", bufs=4))
        singles = ctx.enter_context(tc.tile_pool(name="singles", bufs=1))
        pout = ctx.enter_context(tc.tile_pool(name="pout", bufs=2,
                                              space="PSUM"))

        # gpsimd library providing InstPartitionAllReduce
        nc.gpsimd.load_library(library_config.attn)

        # W4[:, k, :]: bf16 stationary whose only nonzero column is k ->
        # matmul writes chunk-k's neighbor-sum into PSUM partition row k
        # (rows 0-3 contiguous: engine APs require partition step 1, and
        # matmul out base partition must be 0/32/64).  Built in fp32
        # (bf16 memset at 2B-aligned offsets crashes the HW), ACT-cast.
        W4f = singles.tile([P, 4, 4], F32)
        nc.vector.memset(W4f, 0.0)
        for k in range(4):
            nc.vector.memset(W4f[:, k, k:k + 1], 1.0)
        W4 = singles.tile([P, 4, 4], BF16)
        nc.scalar.copy(out=W4, in_=W4f)
        # Prewarm the ACT Square table (~1.3us) before real data arrives
        warm = singles.tile([P, 1], F32)
        nc.vector.memset(warm, 0.0)
        nc.scalar.activation(warm, warm, mybir.ActivationFunctionType.Square)

        def chunks_of(b):
            nq = 4 if b == 0 else 2
            step = A // nq
            return [slice(step * q, step * (q + 1)) for q in range(nq)]

        def load_dma(b):
            X = xp.tile([P, A, IRR, C], F32, tag="X")
            X2 = x2p.tile([P, A, IRR, C], F32, tag="X2")
            Xbf = xbp.tile([P, A, IRR, C], BF16, tag="Xbf")
            for ha in chunks_of(b):
                nc.sync.dma_start(out=X[:, ha], in_=x_v[b][:, ha])
            return X, X2, Xbf

        def do_sq(b, t):
            X, X2, _ = t
            for ha in chunks_of(b):
                nc.scalar.activation(X2[:, ha], X[:, ha],
                                     mybir.ActivationFunctionType.Square)

        def do_cp(b, t):
            X, _, Xbf = t
            for ha in chunks_of(b):
                nc.scalar.copy(out=Xbf[:, ha], in_=X[:, ha])

        def norm_tree(X2, aa):
            """Block norms in place into X2 slots {3,6,9} (l1,l2,l3; l0
            stays slot 0) via a 5-op strided add tree (12 pairwise adds)
            over a-chunk slice `aa`:
              op1: {2,4,6}+={3,5,7}    op2: {10,12,14}+={11,13,15}
              op3: {3,6} = {1,4}+{2,6}  op4: {9,12}+={10,14}
              op5: {6,9}+={8,12}
            """
            def tadd(dst, src):
                nc.vector.tensor_tensor(dst, dst, src, ADD)

            tadd(X2[:, aa, 2:8:2, :], X2[:, aa, 3:9:2, :])
            tadd(X2[:, aa, 10:16:2, :], X2[:, aa, 11:16:2, :])
            nc.vector.tensor_tensor(X2[:, aa, 3:7:3, :],
                                    X2[:, aa, 1:5:3, :],
                                    X2[:, aa, 2:7:4, :], ADD)
            tadd(X2[:, aa, 9:13:3, :], X2[:, aa, 10:15:4, :])
            tadd(X2[:, aa, 6:10:3, :], X2[:, aa, 8:13:4, :])

        def stage1(b, t):
            """Norm tree, a-max, GM."""
            _, X2, _ = t
            if b == 0:
                # per-quarter trees so batch 0's norms start right after
                # quarter 0's square
                for q in range(A):
                    norm_tree(X2, slice(q, q + 1))
            else:
                # per-half trees: tree(b)h0 only needs sq(b)h0, so the DVE
                # can run it inside the ~2.2us window where it would
                # otherwise idle waiting for GM(b-1) + sq(b)h1 (ACT and
                # DVE are phase-locked within ~1us of each other)
                norm_tree(X2, slice(0, 2))
                norm_tree(X2, slice(2, 4))

            # a-max: pairwise over a (0,2),(1,3) then join; norm slots
            # {0,3,6,9} are a single stride-3 AP.
            R = med.tile([P, 2, 4, C], F32, tag="R")
            nc.vector.tensor_tensor(R, X2[:, 0:2, 0:10:3, :],
                                    X2[:, 2:4, 0:10:3, :], MAX)
            M1 = gmp.tile([P, 4, C], F32, tag="M1")
            nc.vector.tensor_tensor(M1, R[:, 0], R[:, 1], MAX)

            # Global max over the 128 partitions, broadcast to all.  [GPSIMD]
            GM = gmp.tile([P, 4, C], F32, tag="GM")
            nc.gpsimd.partition_all_reduce(
                GM.rearrange("p l c -> p (l c)"),
                M1.rearrange("p l c -> p (l c)"),
                channels=P, reduce_op=bass_isa.ReduceOp.max)
            return GM

        def stage2(b, t, GM):
            """Mask, in-place winner-select, PE reduce."""
            _, X2, Xbf = t
            mask = med.tile([P, A, 4, C], BF16, tag="mask")
            nc.vector.tensor_tensor(
                mask, X2[:, :, 0:10:3, :],
                GM.unsqueeze(1).broadcast_to([P, A, 4, C]), EQ)

            Xf = Xbf.rearrange("p a i c -> p a (i c)")
            ps = pout.tile([4, 512], F32, tag="ps")

            def sel(s, e, l):
                nc.vector.tensor_tensor(
                    Xbf[:, :, s:e, :], Xbf[:, :, s:e, :],
                    mask[:, :, l, :].unsqueeze(2).broadcast_to(
                        [P, A, e - s, C]), MULT)

            def mm(k, start=False, stop=False):
                # all 16 matmuls form one accumulation group on ps[4,512]
                # (W4_k zeroes the other rows; every matmul writes all 4)
                for a in range(A):
                    nc.tensor.matmul(ps, W4[:, k, :],
                                     Xf[:, a, k * 512:(k + 1) * 512],
                                     start=(start and a == 0),
                                     stop=(stop and a == A - 1))

            # l3 first so the PE accumulate chains drain early
            sel(9, 16, 3)  # l3: i 9-15
            mm(3, start=True)       # k3 needs i 12-15
            sel(4, 9, 2)   # l2: i 4-8
            mm(2)          # k2 needs i 8-11
            mm(1)          # k1 needs i 4-7
            sel(1, 4, 1)   # l1: i 1-3
            sel(0, 1, 0)   # l0: i 0
            mm(0, stop=True)        # k0 needs i 0-3
            return ps

        def flush(b, ps):
            """PSUM rows 0-3 -> SBUF [4,512] -> DRAM."""
            ob = obp.tile([4, 512], F32, tag="ob")
            nc.scalar.copy(out=ob, in_=ps)
            nc.sync.dma_start(out=out_kv[b], in_=ob)

        # Software pipeline.  ACT runs one batch ahead on squares
        # (sq(b+1) before cp(b)); DVE runs tree(b+1) between amax(b) and
        # mask(b) so the gpsimd global-max latency is hidden.
        tl = {0: load_dma(0), 1: load_dma(1)}
        do_sq(0, tl[0])
        gm = {0: stage1(0, tl[0])}
        do_sq(1, tl[1])
        do_cp(0, tl[0])
        for b in range(B):
            if b + 2 < B:
                tl[b + 2] = load_dma(b + 2)
            if b + 1 < B:
                gm[b + 1] = stage1(b + 1, tl[b + 1])
            if b + 2 < B:
                do_sq(b + 2, tl[b + 2])
            if b + 1 < B:
                do_cp(b + 1, tl[b + 1])
            flush(b, stage2(b, tl.pop(b), gm.pop(b)))

    nc.compile()
    return nc


def kernel(x: np.ndarray, i2l: np.ndarray | None = None) -> np.ndarray:
    x = np.ascontiguousarray(np.asarray(x), dtype=np.float32)
    assert x.shape == (B_FULL, N, IRR, C), x.shape

    if "nc" not in _cache:
        _cache["nc"] = _build_bass()
    nc = _cache["nc"]

    from concourse.bass_utils import run_bass_kernel_spmd

    in_maps = [{"x": x[i * B:(i + 1) * B]} for i in range(N_CORES)]
    res = run_bass_kernel_spmd(nc, in_maps, list(range(N_CORES)))
    out = np.concatenate([res.results[i]["out"] for i in range(N_CORES)], axis=0)
    return out


if __name__ == "__main__":
    xs = np.random.randn(B_FULL, N, IRR, C).astype(np.float32)
    o = kernel(xs)
    print("out", o.shape, o.dtype)
